# revision 9
# baseline (speedup 1.0000x reference)
"""Trainium2 Bass kernel for nn_Decoder_53876069761214 (social-LSTM decoder).

Data-parallel over scenes: 128 scenes of 32 peds -> 16 scenes (512 peds) per
NeuronCore, weights replicated. The per-step social-pooling scatter is a
one-hot matmul on the PE (grid-cell one-hot built on the DVE), followed by the
dense pool matmul accumulated over the 64 grid cells, all in fp32.

Self-contained: hardcodes shapes from the problem spec.
"""
import sys
sys.path.insert(0, "/opt/trn_rl_repo")

import os
import numpy as np
import concourse.bass as bass
import concourse.bacc as bacc
import concourse.mybir as mybir
from concourse.tile import TileContext
from concourse.bass_utils import run_bass_kernel_spmd

F32 = mybir.dt.float32
AF = mybir.ActivationFunctionType
OP = mybir.AluOpType

SEQ_LEN = 12
H = 128
EMB = 64
G = 8
P = 32           # peds per scene
NCORES = 8

MAGIC = 8388608.0   # 2^23, round-to-int trick
SKIP = set(os.environ.get("KBISECT", "").split(","))
SENT = 4096.0       # sentinel added to masked (oob/self) pair cell ids


def build_nc(S_loc=16, steps=SEQ_LEN):
    """Build the per-core Bass program. S_loc scenes of P peds per core."""
    assert S_loc % 4 == 0
    B = P * S_loc          # local peds
    PK = S_loc // 4        # packs of 4 scenes (128 peds each)
    GG = G * G             # 64 cells
    W = PK * P             # pair-tensor width

    nc = bacc.Bacc("TRN2", target_bir_lowering=False, debug=False)

    din = {}
    def dram_in(name, shape):
        din[name] = nc.dram_tensor(name, shape, F32, kind="ExternalInput")
        return din[name]

    for name, shape in [
        ("hT0", [H, B]), ("cT0", [H, B]),
        ("posx0", [1, B]), ("posy0", [1, B]), ("pos4_0", [128, 2 * PK]),
        ("lprx", [1, B]), ("lpry", [1, B]),
        ("wih", [EMB, 4 * H]), ("whh", [H, 4 * H]), ("biasg", [H, 4]),
        ("wh2p", [H, 2]), ("bh2p", [1, 2]),
        ("wemb0", [1, EMB]), ("wemb1", [1, EMB]), ("bemb", [EMB, 1]),
        ("wpool", [H, GG * H]), ("bpool", [H, 1]),
        ("w1", [H, 2 * H]), ("b1", [H, 1]), ("w2", [H, H]), ("b2", [H, 1]),
        ("cellidx2", [128, GG * P * 4]), ("eyec", [128, P]),
        ("ident", [128, 128]), ("ones", [1, 128]), ("bh2p4", [128, 2 * PK]),
    ]:
        dram_in(name, shape)

    out_rel = nc.dram_tensor("out_rel", [steps, 2, B], F32, kind="ExternalOutput")

    with TileContext(nc) as tc:
        with (
            tc.tile_pool(name="const", bufs=1) as cpool,
            tc.tile_pool(name="state", bufs=1) as spool,
            tc.tile_pool(name="work", bufs=2) as work,
            tc.tile_pool(name="mp", bufs=4) as mpool,
            tc.tile_pool(name="ap", bufs=2) as apool,
            tc.tile_pool(name="psg", bufs=1, space="PSUM") as psg,
            tc.tile_pool(name="pss", bufs=1, space="PSUM") as pss,
            tc.tile_pool(name="psp", bufs=1, space="PSUM") as psp,
            tc.tile_pool(name="psr", bufs=1, space="PSUM") as psr,
            tc.tile_pool(name="psmisc", bufs=1, space="PSUM") as psmisc,
        ):
            T = {}
            for name in din:
                if name in ("hT0", "cT0", "posx0", "posy0", "pos4_0",
                            "lprx", "lpry"):
                    continue
                t = cpool.tile(list(din[name].shape), F32, tag=name)
                nc.sync.dma_start(t[:], din[name][:])
                T[name] = t

            # ---- state ----
            hT = spool.tile([H, B], F32, tag="hT")
            cT = spool.tile([H, B], F32, tag="cT")
            xT = spool.tile([EMB, B], F32, tag="xT")
            posx = spool.tile([1, B], F32, tag="posx")
            posy = spool.tile([1, B], F32, tag="posy")
            pos4 = spool.tile([128, 2 * PK], F32, tag="pos4")
            h_nat = spool.tile([128, PK * H], F32, tag="h_nat")
            lprx_sb = spool.tile([1, B], F32, tag="lprx")
            lpry_sb = spool.tile([1, B], F32, tag="lpry")
            for sb_t, dname in [(hT, "hT0"), (cT, "cT0"), (posx, "posx0"),
                                (posy, "posy0"), (pos4, "pos4_0"),
                                (lprx_sb, "lprx"), (lpry_sb, "lpry")]:
                nc.sync.dma_start(sb_t[:], din[dname][:])

            def emb_from(relx_ap, rely_ap):
                """dec_in^T [EMB, B] <- W_emb^T @ rel^T + b_emb, fp32 exact."""
                if "emb" in SKIP:
                    nc.vector.memset(xT[:], 0.01)
                    return
                pe_ = psmisc.tile([EMB, B], F32, tag="misc")
                nc.tensor.matmul(pe_[:], T["wemb0"][:], relx_ap,
                                 start=True, stop=False)
                nc.tensor.matmul(pe_[:], T["wemb1"][:], rely_ap,
                                 start=False, stop=True)
                nc.scalar.activation(xT[:], pe_[:], AF.Identity,
                                     bias=T["bemb"][:, 0:1])

            emb_from(lprx_sb[:], lpry_sb[:])

            gate_fns = [AF.Sigmoid, AF.Sigmoid, AF.Tanh, AF.Sigmoid]

            for t in range(steps):
                # ===== LSTM =====
                gates = []
                for q in range(4):
                    pg = psg.tile([H, B], F32, tag="psgate")
                    nc.tensor.matmul(pg[:], T["wih"][:, q * H:(q + 1) * H],
                                     xT[:], start=True, stop=False)
                    nc.tensor.matmul(pg[:], T["whh"][:, q * H:(q + 1) * H],
                                     hT[:], start=False, stop=True)
                    gq = work.tile([H, B], F32, tag=f"gate{q}")
                    nc.scalar.activation(gq[:], pg[:], gate_fns[q],
                                         bias=T["biasg"][:, q:q + 1])
                    gates.append(gq)
                g_i, g_f, g_g, g_o = gates
                tmp1 = work.tile([H, B], F32, tag="tmp1")
                tmp2 = work.tile([H, B], F32, tag="tmp2")
                nc.vector.tensor_mul(tmp1[:], g_f[:], cT[:])
                nc.vector.tensor_mul(tmp2[:], g_i[:], g_g[:])
                nc.vector.tensor_add(cT[:], tmp1[:], tmp2[:])
                tanh_c = work.tile([H, B], F32, tag="tanhc")
                nc.scalar.activation(tanh_c[:], cT[:], AF.Tanh)
                nc.vector.tensor_mul(hT[:], g_o[:], tanh_c[:])

                # ===== rel + pos update =====
                relx = work.tile([1, B], F32, tag="relx")
                rely = work.tile([1, B], F32, tag="rely")
                if "rel" in SKIP:
                    nc.vector.memset(relx[:], 0.01)
                    nc.vector.memset(rely[:], 0.01)
                else:
                    prx = psr.tile([1, B], F32, tag="relx")
                    pry = psr.tile([1, B], F32, tag="rely")
                    nc.tensor.matmul(prx[:], T["wh2p"][:, 0:1], hT[:],
                                     start=True, stop=True)
                    nc.tensor.matmul(pry[:], T["wh2p"][:, 1:2], hT[:],
                                     start=True, stop=True)
                    nc.vector.tensor_scalar_add(relx[:], prx[:],
                                                T["bh2p"][0:1, 0:1])
                    nc.vector.tensor_scalar_add(rely[:], pry[:],
                                                T["bh2p"][0:1, 1:2])
                nc.sync.dma_start(out_rel[t, 0:1, :], relx[:])
                nc.sync.dma_start(out_rel[t, 1:2, :], rely[:])
                nc.vector.tensor_add(posx[:], posx[:], relx[:])
                nc.vector.tensor_add(posy[:], posy[:], rely[:])

                # pos4 (pair-layout positions) update: rel_nat + bias
                if "relnat" not in SKIP:
                    prn = psmisc.tile([128, 2 * PK], F32, tag="misc")
                    for g in range(PK):
                        nc.tensor.matmul(prn[:, 2 * g:2 * g + 2],
                                         hT[:, g * 128:(g + 1) * 128],
                                         T["wh2p"][:], start=True, stop=True)
                    tmp4 = work.tile([128, 2 * PK], F32, tag="tmp4")
                    nc.vector.tensor_add(tmp4[:], prn[:], T["bh2p4"][:])
                    nc.vector.tensor_add(pos4[:], pos4[:], tmp4[:])

                # ===== next dec_in =====
                emb_from(relx[:], rely[:])

                # ===== h natural layout (scatter lhsT) =====
                if "tp" in SKIP:
                    nc.vector.memset(h_nat[:], 0.01)
                else:
                    pt = psmisc.tile([128, PK * H], F32, tag="misc")
                    for g in range(PK):
                        nc.tensor.transpose(pt[:, g * H:(g + 1) * H],
                                            hT[:, g * 128:(g + 1) * 128],
                                            T["ident"][:])
                    nc.vector.tensor_copy(h_nat[:], pt[:])

                # ===== XB: anchor positions broadcast into pair layout =====
                XB = work.tile([128, 2 * W], F32, tag="XB")
                if "xb" in SKIP:
                    nc.vector.memset(XB[:], 0.01)
                else:
                    pxb = psmisc.tile([128, 2 * W], F32, tag="misc")
                    for g in range(PK):
                        for s in range(4):
                            sc = (g * 4 + s) * P
                            nc.tensor.matmul(
                                pxb[32 * s:32 * s + 32, g * P:(g + 1) * P],
                                T["ones"][0:1, 0:32], posx[0:1, sc:sc + P],
                                start=True, stop=True, tile_position=(0, 32 * s))
                            nc.tensor.matmul(
                                pxb[32 * s:32 * s + 32, W + g * P:W + (g + 1) * P],
                                T["ones"][0:1, 0:32], posy[0:1, sc:sc + P],
                                start=True, stop=True, tile_position=(0, 32 * s))
                    nc.vector.tensor_copy(XB[:], pxb[:])
                XBx = XB[:, 0:W]
                XBy = XB[:, W:2 * W]

                # ===== pair grid indices (batched over packs) =====
                def wtile(tag):
                    return work.tile([128, W], F32, tag=tag, name=tag)
                p4x = work.tile([128, PK], F32, tag="p4x")
                p4y = work.tile([128, PK], F32, tag="p4y")
                pos4_v = pos4[:, :].rearrange("p (g c) -> p c g", c=2)
                nc.vector.tensor_scalar_mul(p4x[:], pos4_v[:, 0, :], 4.0)
                nc.vector.tensor_scalar_mul(p4y[:], pos4_v[:, 1, :], 4.0)
                p4x_bc = p4x[:, :].unsqueeze(2).broadcast_to([128, PK, P])
                p4y_bc = p4y[:, :].unsqueeze(2).broadcast_to([128, PK, P])

                tl4x = wtile("tl4x")
                tl4y = wtile("tl4y")
                nc.vector.tensor_scalar(tl4x[:], XBx, 1.0, 4.0,
                                        op0=OP.subtract, op1=OP.mult)
                nc.vector.tensor_scalar(tl4y[:], XBy, 1.0, 4.0,
                                        op0=OP.add, op1=OP.mult)
                t2x = wtile("t2x")
                t2y = wtile("t2y")
                nc.vector.scalar_tensor_tensor(t2x[:], tl4x[:], -1.0, p4x_bc,
                                               op0=OP.mult, op1=OP.add)
                nc.vector.scalar_tensor_tensor(t2y[:], tl4y[:], 1.0, p4y_bc,
                                               op0=OP.bypass, op1=OP.subtract)
                rx = wtile("rx")
                ry = wtile("ry")
                nc.vector.tensor_scalar(rx[:], t2x[:], MAGIC, MAGIC,
                                        op0=OP.add, op1=OP.subtract)
                nc.vector.tensor_scalar(ry[:], t2y[:], MAGIC, MAGIC,
                                        op0=OP.add, op1=OP.subtract)
                fx = wtile("fx")
                fy = wtile("fy")
                nc.vector.tensor_tensor(fx[:], rx[:], t2x[:], op=OP.is_gt)
                nc.vector.tensor_tensor(fy[:], ry[:], t2y[:], op=OP.is_gt)
                gp = wtile("gp")
                nc.vector.scalar_tensor_tensor(gp[:], ry[:], 8.0, rx[:],
                                               op0=OP.mult, op1=OP.add)
                nc.vector.scalar_tensor_tensor(gp[:], fy[:], -8.0, gp[:],
                                               op0=OP.mult, op1=OP.add)
                nc.vector.tensor_tensor(gp[:], gp[:], fx[:], op=OP.subtract)
                for src, thr, cmp in ((t2x, 0.0, OP.is_le), (t2x, 8.0, OP.is_ge),
                                      (t2y, 0.0, OP.is_le), (t2y, 8.0, OP.is_ge)):
                    mk = wtile("mask")
                    nc.vector.tensor_single_scalar(mk[:], src[:], thr, op=cmp)
                    nc.vector.scalar_tensor_tensor(gp[:], mk[:], SENT, gp[:],
                                                   op0=OP.mult, op1=OP.add)
                eye_bc = T["eyec"][:, :].unsqueeze(1).broadcast_to([128, PK, P])
                nc.vector.tensor_tensor(gp[:], gp[:], eye_bc, op=OP.add)

                # ===== scatter + A copies + pool matmul =====
                pool_h = work.tile([H, B], F32, tag="poolh")
                POOLSBUF = "poolsbuf" in SKIP
                if "scatter" in SKIP:
                    nc.vector.memset(pool_h[:], 0.01)
                else:
                  if POOLSBUF:
                      pool_acc = work.tile([H, B], F32, tag="poolacc")
                  else:
                      pspool = psp.tile([H, B], F32, tag="pspool")
                  for e in range(8):                       # cell-eighths
                    a_unit = apool.tile([128, 8 * B], F32, tag="asb")
                    for g in range(PK):
                        M2t = mpool.tile([128, 1024], F32, tag="M2",
                                         name="M2t")
                        gp_bc = gp[:, g * P:(g + 1) * P].unsqueeze(1) \
                            .unsqueeze(1).broadcast_to([128, 8, 4, P])
                        nc.vector.tensor_tensor(
                            M2t[:], gp_bc,
                            T["cellidx2"][:, e * 1024:(e + 1) * 1024],
                            op=OP.is_equal)
                        psA = pss.tile([128, 1024], F32, tag="psA")
                        for hf in range(2):
                            nc.tensor.matmul(psA[:, hf * 512:(hf + 1) * 512],
                                             h_nat[:, g * H:(g + 1) * H],
                                             M2t[:, hf * 512:(hf + 1) * 512],
                                             start=True, stop=True)
                        src = psA[:, :].rearrange("p (c s b) -> p c s b",
                                                  c=8, s=4)
                        dst = a_unit[:, :].rearrange(
                            "p (c s b) -> p c s b", c=8, s=S_loc
                        )[:, :, g * 4:(g + 1) * 4, :]
                        if g % 2 == 0:
                            nc.vector.tensor_copy(dst, src)
                        else:
                            nc.scalar.copy(dst, src)
                    if POOLSBUF:
                        pse = psp.tile([H, B], F32, tag="pspool")
                        for cl in range(8):
                            c = e * 8 + cl
                            nc.tensor.matmul(pse[:],
                                             T["wpool"][:, c * H:(c + 1) * H],
                                             a_unit[:, cl * B:(cl + 1) * B],
                                             start=(cl == 0), stop=(cl == 7))
                        if e == 0:
                            nc.vector.tensor_copy(pool_acc[:], pse[:])
                        else:
                            nc.vector.tensor_add(pool_acc[:], pool_acc[:], pse[:])
                    else:
                        for cl in range(8):
                            c = e * 8 + cl
                            nc.tensor.matmul(pspool[:],
                                             T["wpool"][:, c * H:(c + 1) * H],
                                             a_unit[:, cl * B:(cl + 1) * B],
                                             start=(c == 0), stop=(c == GG - 1))
                  nc.scalar.activation(pool_h[:],
                                       pool_acc[:] if POOLSBUF else pspool[:],
                                       AF.Relu, bias=T["bpool"][:, 0:1])

                # ===== MLP =====
                pm1 = psmisc.tile([H, B], F32, tag="misc")
                nc.tensor.matmul(pm1[:], T["w1"][:, 0:H], hT[:],
                                 start=True, stop=False)
                nc.tensor.matmul(pm1[:], T["w1"][:, H:2 * H], pool_h[:],
                                 start=False, stop=True)
                m1 = work.tile([H, B], F32, tag="m1")
                nc.scalar.activation(m1[:], pm1[:], AF.Relu,
                                     bias=T["b1"][:, 0:1])
                pm2 = psmisc.tile([H, B], F32, tag="misc")
                nc.tensor.matmul(pm2[:], T["w2"][:], m1[:],
                                 start=True, stop=True)
                nc.scalar.activation(hT[:], pm2[:], AF.Relu,
                                     bias=T["b2"][:, 0:1])

    nc.compile()
    return nc


def host_prep(inputs, S_loc=16, n_cores=NCORES):
    """Split full inputs into per-core input maps."""
    lp = np.asarray(inputs["last_pos"], np.float32)
    lpr = np.asarray(inputs["last_pos_rel"], np.float32)
    h0 = np.asarray(inputs["h0"], np.float32)
    c0 = np.asarray(inputs["c0"], np.float32)
    B_loc = P * S_loc
    PK = S_loc // 4
    GG = G * G

    f = lambda k: np.asarray(inputs[k], np.float32)
    W_emb, b_emb = f("W_emb"), f("b_emb")
    W_ih, W_hh, b_ih, b_hh = f("W_ih"), f("W_hh"), f("b_ih"), f("b_hh")
    W_h2p, b_h2p = f("W_h2p"), f("b_h2p")
    W_pool, b_pool = f("W_pool"), f("b_pool")
    W1, b1, W2, b2 = f("W1"), f("b1"), f("W2"), f("b2")

    biasg = np.ascontiguousarray((b_ih + b_hh).reshape(4, H).T)
    wpool_dev = np.ascontiguousarray(
        W_pool.reshape(GG, H, H).transpose(1, 0, 2).reshape(H, GG * H))
    cellidx2 = np.full((128, GG * 4 * P), -1.0, np.float32)
    col_c = (np.arange(GG * 4 * P) // (4 * P)).astype(np.int64)
    col_s = (np.arange(GG * 4 * P) // P) % 4
    for p_ in range(128):
        cellidx2[p_, col_s == (p_ // P)] = col_c[col_s == (p_ // P)]
    eyec = np.zeros((128, P), np.float32)
    for p_ in range(128):
        eyec[p_, p_ % P] = SENT
    ident = np.eye(128, dtype=np.float32)
    ones = np.ones((1, 128), np.float32)
    bh2p4 = np.ascontiguousarray(
        np.tile(b_h2p.reshape(1, 2), (128, PK)).astype(np.float32))

    shared = dict(
        wih=W_ih, whh=W_hh, biasg=biasg, wh2p=W_h2p,
        bh2p=np.ascontiguousarray(b_h2p.reshape(1, 2)),
        wemb0=np.ascontiguousarray(W_emb[0:1, :]),
        wemb1=np.ascontiguousarray(W_emb[1:2, :]),
        bemb=np.ascontiguousarray(b_emb.reshape(EMB, 1)),
        wpool=wpool_dev, bpool=np.ascontiguousarray(b_pool.reshape(H, 1)),
        w1=np.ascontiguousarray(np.concatenate([W1[0:H, :], W1[H:2 * H, :]],
                                               axis=1)),
        b1=np.ascontiguousarray(b1.reshape(H, 1)),
        w2=W2, b2=np.ascontiguousarray(b2.reshape(H, 1)),
        cellidx2=cellidx2, eyec=eyec, ident=ident, ones=ones, bh2p4=bh2p4,
    )

    in_maps = []
    for core in range(n_cores):
        sl = slice(core * B_loc, (core + 1) * B_loc)
        pos = lp[sl]
        pos4_0 = np.zeros((128, 2 * PK), np.float32)
        for g in range(PK):
            blk = pos[g * 128:(g + 1) * 128]
            pos4_0[:, 2 * g] = blk[:, 0]
            pos4_0[:, 2 * g + 1] = blk[:, 1]
        m = dict(shared)
        m.update(
            hT0=np.ascontiguousarray(h0[sl].T),
            cT0=np.ascontiguousarray(c0[sl].T),
            posx0=np.ascontiguousarray(pos[:, 0].reshape(1, B_loc)),
            posy0=np.ascontiguousarray(pos[:, 1].reshape(1, B_loc)),
            pos4_0=pos4_0,
            lprx=np.ascontiguousarray(lpr[sl, 0].reshape(1, B_loc)),
            lpry=np.ascontiguousarray(lpr[sl, 1].reshape(1, B_loc)),
        )
        in_maps.append(m)
    return in_maps


_CACHED_NC = {}


def kernel(**inputs) -> np.ndarray:
    num_ped = int(inputs["num_ped"])
    assert num_ped == P, f"kernel hardcoded for num_ped=32, got {num_ped}"
    B = np.asarray(inputs["h0"]).shape[0]
    assert B == 4096
    S_loc = (B // P) // NCORES

    key = (S_loc, SEQ_LEN)
    if key not in _CACHED_NC:
        _CACHED_NC[key] = build_nc(S_loc=S_loc, steps=SEQ_LEN)
    nc = _CACHED_NC[key]

    in_maps = host_prep(inputs, S_loc=S_loc)
    res = run_bass_kernel_spmd(nc, in_maps, core_ids=list(range(NCORES)))

    B_loc = P * S_loc
    out = np.empty((SEQ_LEN, B, 2), np.float32)
    for core in range(NCORES):
        r = res.results[core]["out_rel"]        # [steps, 2, B_loc]
        out[:, core * B_loc:(core + 1) * B_loc, 0] = r[:, 0, :]
        out[:, core * B_loc:(core + 1) * B_loc, 1] = r[:, 1, :]
    return out
